# revision 1
# baseline (speedup 1.0000x reference)
"""Trainium2 Bass kernel for nn_DeformAttn (sparse per-pixel attention).

Computation (per batch b, H=8 heads x 16 ch, S=9 samples, D=16384 pixels):
  qp = Wq@q + bq ; kp = Wk@kv ; vp = Wv@kv
  logits[h,s,d] = sum_c_in_head (qp+bq)[c,d] * (kp+bk)[c,d] * 0.25
  attn = softmax_s(logits); out[c,d] = sum_s attn[h(c),s,d] * (vp+bv)[c,d]

Sharding: 8 cores = batch(4) x spatial-half(2). No collectives.

Per-core dataflow (16 tiles of 512 pixels), all matmuls N=512:
  - projections as float32r matmuls (1 cyc/row)
  - per-head logit reduction via constant 0/1 bf16 mask matmuls channels->72
    (head,sample) partition rows; bk bias folded in via a bk-mask matmul
    (note sum_c qp'*(kp+bk) = sum qp'kp + sum qp'bk with qp' = qp+bq)
  - softmax over s without max-subtraction (logits ~ +-6): ACT exp,
    sum-over-s + 1/Z broadcast via tiny mask matmuls, reciprocal_approx_fast
  - value phase: PE expands attn [72,N]->[128,N] per sample via bf16 mask
    matmuls; DVE multiplies vs vp, bf16 adds-tree over s; bv added at the
    final ScalarE bias-copy (valid since sum_s attn = 1)
  - DVE/ACT balanced: for s in KP_ACT/AE_ACT the PSUM drain goes through a
    ScalarE bf16 copy so the DVE tensor_tensor runs at 2x; other samples are
    consumed directly from PSUM at 1x.

Hardware constraint that shapes the emission order: every instruction can
carry at most ONE semaphore wait (bf16 matmuls split LDW+MM and so get two
slots; fp32/fp32r self-loading matmuls get one total). The loop is ordered
so each instruction needs at most one NEW semaphore under Tile's
vector-clock subsumption; dummy ldweights/copy "joins" pre-absorb the
constant-blob DMA semaphores, and output staging tiles are dedicated
(bufs=NT) so the store path has no write-after-read waits.
"""
import os
import sys

for _p in ("/opt/trn_rl_repo", "/root/.axon_site/_ro/trn_rl_repo"):
    if os.path.isdir(_p) and _p not in sys.path:
        sys.path.insert(0, _p)

import numpy as np
import ml_dtypes
from contextlib import ExitStack

import concourse.bass as bass
import concourse.bacc as bacc
import concourse.tile as tile
from concourse import mybir
from concourse.bass_utils import run_bass_kernel_spmd

F32 = mybir.dt.float32
F32R = mybir.dt.float32r
BF16 = mybir.dt.float16  # fp16: 10-bit mantissa, same PE/DVE speed as bf16
AF = mybir.ActivationFunctionType

B, C = 4, 128
H, HC, S = 8, 16, 9
FH, FW = 128, 128
D_FULL = FH * FW          # 16384
D_HALF = D_FULL // 2      # 8192 pixels per core
TN = 512                  # pixels per tile
NT = D_HALF // TN         # 16 tiles
SCALE = HC ** -0.5        # 0.25
N_CORES = 8

# samples whose PSUM drain routes through ScalarE (bf16 copy -> DVE 2x TT).
# Must be a prefix {0..k} so the first DVE op of each group carries the one
# allowed ACT wait and later direct-PSUM ops inherit the ACT clock.
KP_ACT = frozenset({0, 1, 2})
AE_ACT = frozenset({0, 1, 2, 3, 4})

# bf16 const blob column layout
KM_OFF = 0                 # kmask: S slices of [128, 72]
BKM_OFF = KM_OFF + S * 72  # bkmask [128, 72]
EM_OFF = BKM_OFF + 72      # emask: S slices of [72, 128]
GM_OFF = EM_OFF + S * 128  # gmask [72, 8]
IM_OFF = GM_OFF + 8        # identity [128, 128]
NB = IM_OFF + 128          # bf16 cols

# f32 const blob column layout: bq | bv | b72 [8, 72]
NF = 2 + 72


def _build_nc(repeat=1):
    nc = bacc.Bacc("TRN2", target_bir_lowering=False, debug=False,
                   num_devices=N_CORES)
    dp = nc.declare_dram_parameter
    q_d = dp("q", [C, D_HALF], F32R, isOutput=False)
    kv_d = dp("kv", [C, NT, S, TN], F32R, isOutput=False)
    wr_d = dp("blob_r", [C, 3 * C], F32R, isOutput=False)   # WqT|WkT|WvT
    bb_d = dp("blob_b", [C, NB], BF16, isOutput=False)
    bf_d = dp("blob_f", [C, NF], F32, isOutput=False)
    out_d = dp("out", [C, D_HALF], F32, isOutput=True)

    with ExitStack() as ctx:
        tc = ctx.enter_context(tile.TileContext(nc))
        p_const = ctx.enter_context(tc.tile_pool(name="consts", bufs=1))
        p_qin = ctx.enter_context(tc.tile_pool(name="qin", bufs=4))
        p_kvin = ctx.enter_context(tc.tile_pool(name="kvin", bufs=4))
        p_qp = ctx.enter_context(tc.tile_pool(name="qp", bufs=3))
        p_kpbf = ctx.enter_context(tc.tile_pool(name="kpbf", bufs=6))
        p_t = ctx.enter_context(tc.tile_pool(name="t", bufs=6))
        p_vp = ctx.enter_context(tc.tile_pool(name="vp", bufs=3))
        p_sm = ctx.enter_context(tc.tile_pool(name="sm", bufs=3))
        p_aebf = ctx.enter_context(tc.tile_pool(name="aebf", bufs=6))
        p_u = ctx.enter_context(tc.tile_pool(name="u", bufs=6))
        p_out = ctx.enter_context(tc.tile_pool(name="outp", bufs=NT))
        ps_kp = ctx.enter_context(tc.tile_pool(name="pskp", bufs=2, space="PSUM"))
        ps_vp = ctx.enter_context(tc.tile_pool(name="psvp", bufs=2, space="PSUM"))
        ps_at = ctx.enter_context(tc.tile_pool(name="psat", bufs=1, space="PSUM"))
        ps_ae = ctx.enter_context(tc.tile_pool(name="psae", bufs=2, space="PSUM"))
        ps_o = ctx.enter_context(tc.tile_pool(name="pso", bufs=1, space="PSUM"))

        # ---- constants (one DMA per blob) ----
        wr_sb = p_const.tile([C, 3 * C], F32R)
        nc.sync.dma_start(wr_sb[:], wr_d[:])
        bb_sb = p_const.tile([C, NB], BF16)
        nc.sync.dma_start(bb_sb[:], bb_d[:])
        bf_sb = p_const.tile([C, NF], F32)
        nc.sync.dma_start(bf_sb[:], bf_d[:])

        # joins: let PE/ACT observe each const-DMA queue up front so later
        # 1-wait-limited instructions only wait on their streaming operand
        nc.tensor.ldweights(bb_sb[:, 0:128])
        nc.tensor.ldweights(wr_sb[:, 0:64].bitcast(BF16))
        nc.tensor.ldweights(bf_sb[:, 0:64].bitcast(BF16))
        act_join = p_const.tile([C, 1], F32)
        nc.scalar.copy(act_join[:], bf_sb[:, 0:1])
        dve_join = p_const.tile([C, 1], F32)
        nc.vector.tensor_copy(dve_join[:], bf_sb[:, 0:1])

        wq_t = wr_sb[:, 0:C]
        wk_t = wr_sb[:, C:2 * C]
        wv_t = wr_sb[:, 2 * C:3 * C]
        bq_col = bf_sb[:, 0:1]
        bv_col = bf_sb[:, 1:2]
        b72 = bf_sb[0:8, 2:2 + 72]
        bkm = bb_sb[:, BKM_OFF:BKM_OFF + 72]
        gm = bb_sb[0:72, GM_OFF:GM_OFF + 8]
        im = bb_sb[:, IM_OFF:IM_OFF + 128]

        def _body():
          for t in range(NT):
              # ---- loads ----
              q_t = p_qin.tile([C, TN], F32R)
              nc.sync.dma_start(q_t[:], q_d[:, t * TN:(t + 1) * TN])
              kv_t = p_kvin.tile([C, S, TN], F32R)
              nc.sync.dma_start(kv_t[:], kv_d[:, t])

              # ---- q projection + bias (fp32 and bf16 twins) ----
              qp_ps = ps_kp.tile([C, TN], F32, tag="kp_ps")
              nc.tensor.matmul(qp_ps[:], wq_t, q_t[:], start=True, stop=True)
              qp_bf = p_qp.tile([C, TN], BF16, tag="qpb")
              nc.scalar.activation(qp_bf[:], qp_ps[:], AF.Identity, bias=bq_col)

              # ---- k projections, t = qp*kp, logit mask-matmuls ----
              lg_ps = ps_at.tile([72, TN], F32, tag="at")
              kp_list = []
              for s in range(min(2, S)):
                  kp_ps = ps_kp.tile([C, TN], F32)
                  nc.tensor.matmul(kp_ps[:], wk_t, kv_t[:, s], start=True, stop=True)
                  kp_list.append(kp_ps)
              vp_bf = p_vp.tile([C, S, TN], BF16)
              for s in range(S):
                  if s + 2 < S:
                      kp_ps = ps_kp.tile([C, TN], F32)
                      nc.tensor.matmul(kp_ps[:], wk_t, kv_t[:, s + 2],
                                       start=True, stop=True)
                      kp_list.append(kp_ps)
                  kp_ps = kp_list[s]
                  t_sb = p_t.tile([C, TN], BF16)
                  if s in KP_ACT:
                      kp_bf = p_kpbf.tile([C, TN], BF16)
                      nc.scalar.copy(kp_bf[:], kp_ps[:])
                      nc.vector.tensor_mul(t_sb[:], qp_bf[:], kp_bf[:])
                  else:
                      nc.vector.tensor_mul(t_sb[:], qp_bf[:], kp_ps[:])
                  nc.tensor.matmul(
                      lg_ps[:], bb_sb[:, KM_OFF + s * 72:KM_OFF + (s + 1) * 72],
                      t_sb[:], start=(s == 0), stop=False,
                  )
                  # interleave v projections + drains: keeps ScalarE dense
                  # during the t-mult phase (vp 0..5 here, 6..8 as PE filler
                  # for the softmax latency chain below)
                  if s < 6:
                      vp_ps = ps_vp.tile([C, TN], F32)
                      nc.tensor.matmul(vp_ps[:], wv_t, kv_t[:, s],
                                       start=True, stop=True)
                      nc.scalar.copy(vp_bf[:, s], vp_ps[:])
              # bias term: sum_c qp'[c]*bk[c] per head (same for all s)
              nc.tensor.matmul(lg_ps[:], bkm, qp_bf[:], start=False, stop=True)

              # ---- softmax over s (no max-subtraction; logits bounded) ----
              exp_sb = p_sm.tile([72, TN], BF16, tag="exp")
              nc.scalar.activation(exp_sb[:], lg_ps[:], AF.Exp, scale=SCALE)

              z_ps = ps_at.tile([8, TN], F32, tag="at")
              nc.tensor.matmul(z_ps[:], gm, exp_sb[:], start=True, stop=True)

              for s in range(6, S):
                  vp_ps = ps_vp.tile([C, TN], F32)
                  nc.tensor.matmul(vp_ps[:], wv_t, kv_t[:, s], start=True, stop=True)
                  if s == 8:
                      nc.vector.tensor_copy(vp_bf[:, s], vp_ps[:])
                  else:
                      nc.scalar.copy(vp_bf[:, s], vp_ps[:])

              rz_sb = p_sm.tile([8, TN], F32, tag="rz")
              nc.vector.reciprocal_approx_fast(out=rz_sb[:], in_=z_ps[:])
              zb_ps = ps_at.tile([72, TN], F32, tag="at")
              nc.tensor.matmul(zb_ps[:], b72, rz_sb[:], start=True, stop=True)
              # join: absorb the ACT(exp) wait so attn below needs only PE(zb)
              ej_sb = p_sm.tile([8, 1], BF16, tag="ej")
              nc.vector.tensor_copy(ej_sb[:], exp_sb[0:8, 0:1])
              attn_sb = p_sm.tile([72, TN], BF16, tag="attn")
              nc.vector.tensor_mul(attn_sb[:], exp_sb[:], zb_ps[:])

              # ---- expand attn per sample; u = vp * attn_expanded;
              # sum over s on PE via identity-matmul accumulation ----
              o_ps = ps_o.tile([C, TN], F32)
              for s in range(S):
                  ae_ps = ps_ae.tile([C, TN], F32)
                  nc.tensor.matmul(
                      ae_ps[:], bb_sb[0:72, EM_OFF + s * 128:EM_OFF + (s + 1) * 128],
                      attn_sb[:], start=True, stop=True,
                  )
                  u_sb = p_u.tile([C, TN], BF16)
                  if s in AE_ACT:
                      ae_bf = p_aebf.tile([C, TN], BF16)
                      nc.scalar.copy(ae_bf[:], ae_ps[:])
                      nc.vector.tensor_mul(u_sb[:], vp_bf[:, s], ae_bf[:])
                  else:
                      nc.vector.tensor_mul(u_sb[:], vp_bf[:, s], ae_ps[:])
                  nc.tensor.matmul(o_ps[:], im, u_sb[:],
                                   start=(s == 0), stop=(s == S - 1))

              out_sb = p_out.tile([C, TN], F32)
              nc.scalar.activation(out_sb[:], o_ps[:], AF.Identity, bias=bv_col)
              nc.sync.dma_start(out_d[:, t * TN:(t + 1) * TN], out_sb[:])
        if repeat == 1:
            _body()
        else:
            with tc.For_i(0, repeat, 1):
                _body()
    nc.compile()
    return nc


def _make_consts(Wq, bq, Wk, bk, Wv, bv):
    bf = np.float16
    blob_r = np.concatenate(
        [np.ascontiguousarray(Wq.T), np.ascontiguousarray(Wk.T),
         np.ascontiguousarray(Wv.T)], axis=1
    ).astype(np.float32)

    blob_b = np.zeros((C, NB), dtype=bf)
    cc = np.arange(C)
    km = np.zeros((C, 72), dtype=np.float32)
    for s in range(S):
        km[:] = 0.0
        km[cc, (cc // HC) * S + s] = 1.0
        blob_b[:, KM_OFF + s * 72:KM_OFF + (s + 1) * 72] = km.astype(bf)
    bkm = np.zeros((C, 72), dtype=np.float32)
    for s in range(S):
        bkm[cc, (cc // HC) * S + s] = bk
    blob_b[:, BKM_OFF:BKM_OFF + 72] = bkm.astype(bf)
    mm = np.arange(C)
    em = np.zeros((72, C), dtype=np.float32)
    for s in range(S):
        em[:] = 0.0
        em[(mm // HC) * S + s, mm] = 1.0
        blob_b[0:72, EM_OFF + s * 128:EM_OFF + (s + 1) * 128] = em.astype(bf)
    gmask = np.zeros((72, 8), dtype=np.float32)
    jj = np.arange(72)
    gmask[jj, jj // S] = 1.0
    blob_b[0:72, GM_OFF:GM_OFF + 8] = gmask.astype(bf)
    blob_b[:, IM_OFF:IM_OFF + 128] = np.eye(C, dtype=np.float32).astype(bf)

    blob_f = np.zeros((C, NF), dtype=np.float32)
    blob_f[:, 0] = bq
    blob_f[:, 1] = bv
    b72 = np.zeros((8, 72), dtype=np.float32)
    b72[jj // S, jj] = 1.0
    blob_f[0:8, 2:2 + 72] = b72
    return blob_r, blob_b, blob_f


_NC_CACHE = []


def _make_in_maps(q, kv, Wq, bq, Wk, bk, Wv, bv):
    blob_r, blob_b, blob_f = _make_consts(Wq, bq, Wk, bk, Wv, bv)
    q_flat = q.reshape(B, C, D_FULL)
    kv_flat = kv.reshape(B, C, S, D_FULL)
    in_maps = []
    for core in range(N_CORES):
        b = core // 2
        half = core % 2
        sl = slice(half * D_HALF, (half + 1) * D_HALF)
        q_sh = np.ascontiguousarray(q_flat[b, :, sl])
        kv_sh = np.ascontiguousarray(
            kv_flat[b, :, :, sl].reshape(C, S, NT, TN).transpose(0, 2, 1, 3)
        )                                                  # [C, NT, S, TN]
        in_maps.append({
            "q": q_sh, "kv": kv_sh,
            "blob_r": blob_r, "blob_b": blob_b, "blob_f": blob_f,
        })
    return in_maps


def kernel(q, kv, Wq, bq, Wk, bk, Wv, bv):
    q = np.asarray(q, dtype=np.float32)
    kv = np.asarray(kv, dtype=np.float32)
    args = [np.asarray(a, dtype=np.float32) for a in (Wq, bq, Wk, bk, Wv, bv)]
    in_maps = _make_in_maps(q, kv, *args)

    if not _NC_CACHE:
        _NC_CACHE.append(_build_nc())
    nc = _NC_CACHE[0]
    res = run_bass_kernel_spmd(nc, in_maps, list(range(N_CORES)))

    out = np.empty((B, C, D_FULL), dtype=np.float32)
    for core in range(N_CORES):
        b = core // 2
        half = core % 2
        out[b, :, half * D_HALF:(half + 1) * D_HALF] = res.results[core]["out"]
    return out.reshape(B, C, FH, FW)


if __name__ == "__main__":
    rng = np.random.default_rng(0)
    ins = {
        "q": rng.standard_normal((B, C, FH, FW), dtype=np.float32),
        "kv": rng.standard_normal((B, C, S, D_FULL), dtype=np.float32),
        "Wq": rng.standard_normal((C, C), dtype=np.float32) * C ** -0.5,
        "bq": (rng.standard_normal(C) * 0.01).astype(np.float32),
        "Wk": rng.standard_normal((C, C), dtype=np.float32) * C ** -0.5,
        "bk": (rng.standard_normal(C) * 0.01).astype(np.float32),
        "Wv": rng.standard_normal((C, C), dtype=np.float32) * C ** -0.5,
        "bv": (rng.standard_normal(C) * 0.01).astype(np.float32),
    }
    out = kernel(**ins)
    print("ran, out shape", out.shape, "finite:", np.isfinite(out).all())

